# revision 27
# baseline (speedup 1.0000x reference)
"""Trainium2 Bass kernel for nn_Net_76270029242478 (gnn_message_passing).

Math (B=32, N=100, E=256, H=1024, MID=256):
  t        = einsum('bije,em->bijm', trans_mat, W_r) + b_r
  qp       = q @ W_q + b_q
  relation = einsum('bijm,m->bij', t * qp[:,None,None,:], W_out[:,0]) + b_out
  relation = where(r_mask==0, -inf, relation); softmax over i (axis=1)
  out      = einsum('bij,bj->bi', softmax, z_logits)

Algebraic fold (exact):
  relation[b,i,j] = trans_mat[b,i,j,:] . u[b,:] + c[b]
    u[b,e] = sum_m W_r[e,m] * (qp[b,m]+b_q[m]) * W_out[m,0]
  c[b] is constant over (i,j) so it cancels in the softmax over i.

Device strategy (v5):
  - Host pre-transposes trans_mat to [b, e, i, j]: the device streams it with
    e on partitions as fully contiguous 4-20 KB descriptor runs across all
    128 partitions -> all 16 SDMA engines at the ~358 GB/s HBM roofline.
  - Stream DMAs cast f32 -> bf16 in flight (SWDGE/gpsimd) so PE weight loads
    run at 1 cycle/column (fp32 would be 4).
  - ALL small tensors (weights, q, biases, z, f32-converted mask) are packed
    on the host into TWO [128, X] blocks loaded with two DMAs with fat
    descriptors: they land in a couple of us even while the stream hogs the
    SDMA engines, and no device-side cast/unpack op exists to tangle the
    engine programs.
  - The first i-rows of sample 0 are loaded as f32 on the HWDGE rings (ready
    ~2 us before the SWDGE path) and cast on the (otherwise idle) DVE,
    hiding the SWDGE warmup.
  - rel is computed on the TensorEngine as a batched mat-vec over e:
    psum[j, i] += T[e, i, j]^T u[b, e], two 128-e halves per column.
  - Softmax lands in [j_part, i_free] layout: exp (ACT), mask-mult +
    denominator (one DVE op with accum), final aggregation is one matmul.
  - The last sample's chunks taper so the post-stream tail is short.

Sharding: data-parallel over batch, 4 samples per core x 8 cores.
"""

import ml_dtypes
import numpy as np

import concourse.bass as bass
import concourse.tile as tile
from concourse import bacc, mybir
from concourse.bass_utils import run_bass_kernel_spmd

F32 = mybir.dt.float32
BF16 = mybir.dt.bfloat16
Alu = mybir.AluOpType
ActF = mybir.ActivationFunctionType

B, N, E, H, MID = 32, 100, 256, 1024, 256
NCORES = 8
BPC = B // NCORES       # samples per core = 4
EH = E // 128           # 2 e-halves (contraction chunks)
HK = H // 128           # 8 contraction chunks for q @ W_q
MK = MID // 128         # 2 contraction chunks
WARM = 10               # i-rows of sample 0 loaded via HWDGE f32 warm-start
# i-row chunk schedule per sample (SWDGE bf16 cast stream)
CHUNKS = {
    0: [(WARM, 20), (30, 20), (50, 25), (75, 25)],  # 0:10 = the warm pair
    1: [(0, 25), (25, 25), (50, 25), (75, 25)],
    2: [(0, 25), (25, 25), (50, 25), (75, 25)],
    # taper the last sample so the post-stream cast+matvec drain is short
    3: [(0, 25), (25, 25), (50, 25), (75, 15), (90, 6), (96, 4)],
}
# packA (bf16) column offsets
A_WQ, A_WR, A_Q = 0, HK * MID, HK * MID + MK * E
A_W = A_Q + HK * BPC                     # 2592
# packB (f32) column offsets
B_BW, B_Z, B_MASK = 0, 2 * MK, 2 * MK + BPC
B_W = B_MASK + BPC * N                   # 408


def _build():
    nc = bacc.Bacc("TRN2", target_bir_lowering=False, debug=False,
                   num_devices=NCORES)

    # trans pre-transposed on host to [b, e, i, j] (e on partitions)
    transT_d = nc.declare_dram_parameter("transT", [BPC, E, N, N], F32,
                                         isOutput=False)
    packA_d = nc.declare_dram_parameter("packA", [128, A_W], BF16, isOutput=False)
    packB_d = nc.declare_dram_parameter("packB", [128, B_W], F32, isOutput=False)
    outT_d = nc.declare_dram_parameter("outT", [N, BPC], F32, isOutput=True)

    with tile.TileContext(nc) as tc, \
         tc.tile_pool(name="const", bufs=1) as const_pool, \
         tc.tile_pool(name="stream", bufs=8) as stream_pool, \
         tc.tile_pool(name="swf32", bufs=6) as swf32_pool, \
         tc.tile_pool(name="warm", bufs=2) as warm_pool, \
         tc.tile_pool(name="epi", bufs=6) as epi_pool, \
         tc.tile_pool(name="psum_rel", bufs=2, space="PSUM") as psum_rel, \
         tc.tile_pool(name="psum_sm", bufs=2, space="PSUM") as psum_sm:

        # ---------- consts first (fat descriptors, land in ~2-3 us) --------
        pA = const_pool.tile([128, A_W], BF16)
        nc.sync.dma_start(pA[:], packA_d[:])
        pB = const_pool.tile([128, B_W], F32)
        nc.scalar.dma_start(pB[:], packB_d[:])

        # ---------- warm-start: rows 0:WARM of sample 0 as f32 on HWDGE ----
        warm_f32 = []
        for h in range(EH):
            wt = warm_pool.tile([128, WARM, N], F32)
            eng = nc.sync if h == 0 else nc.scalar
            eng.dma_start(wt[:], transT_d[0, h * 128:(h + 1) * 128, 0:WARM, :])
            warm_f32.append(wt)
        warm_bf = []
        for h in range(EH):
            wb = stream_pool.tile([128, WARM, N], BF16)
            nc.vector.tensor_copy(wb[:], warm_f32[h][:])
            warm_bf.append(wb)

        # ---------- prologue: u[b,e] with e on partitions, bf16 ----------
        # qpT[m,b] = sum_h W_q[h,m] * q[b,h]
        vT_sb = const_pool.tile([128, MK, BPC], BF16)
        for mk in range(MK):
            qpT_ps = psum_sm.tile([128, BPC], F32)
            for hk in range(HK):
                nc.tensor.matmul(
                    qpT_ps[:],
                    pA[:, A_WQ + hk * MID + mk * 128:A_WQ + hk * MID + (mk + 1) * 128],
                    pA[:, A_Q + hk * BPC:A_Q + (hk + 1) * BPC],
                    start=(hk == 0), stop=(hk == HK - 1),
                )
            # vT[m,b] = (qpT[m,b] + b_q[m]) * W_out[m]
            nc.vector.tensor_scalar(
                out=vT_sb[:, mk, :], in0=qpT_ps[:],
                scalar1=pB[:, B_BW + mk:B_BW + mk + 1],
                scalar2=pB[:, B_BW + MK + mk:B_BW + MK + mk + 1],
                op0=Alu.add, op1=Alu.mult,
            )

        # uT[e', h, b] = sum_m W_r[128h+e', m] * vT[m, b]  (partition = e')
        uTb_sb = const_pool.tile([128, EH, BPC], BF16)
        for h in range(EH):
            uT_ps = psum_sm.tile([128, BPC], F32)
            for mk in range(MK):
                nc.tensor.matmul(
                    uT_ps[:],
                    pA[:, A_WR + mk * E + h * 128:A_WR + mk * E + (h + 1) * 128],
                    vT_sb[:, mk, :],
                    start=(mk == 0), stop=(mk == MK - 1),
                )
            nc.vector.tensor_copy(uTb_sb[:, h, :], uT_ps[:])

        outT_sb = const_pool.tile([N, BPC], F32)

        # ---------- main stream + batched mat-vec on the PE ----------
        def matvec_block(psum_b, b, ch_pair, i0, ib):
            for il in range(ib):
                i = i0 + il
                nc.tensor.matmul(psum_b[:, i:i + 1], ch_pair[0][:, il, :],
                                 uTb_sb[:, 0, b:b + 1], start=True, stop=False)
                nc.tensor.matmul(psum_b[:, i:i + 1], ch_pair[1][:, il, :],
                                 uTb_sb[:, 1, b:b + 1], start=False, stop=True)

        def sample(b):
            psum_b = psum_rel.tile([N, N], F32)
            if b == 0:
                matvec_block(psum_b, 0, warm_bf, 0, WARM)
            for (i0, ib) in CHUNKS[b]:
                ch = []
                for h in range(EH):
                    # SWDGE f32 load (no in-DMA cast), bf16 cast on the DVE
                    ft = swf32_pool.tile([128, ib, N], F32)
                    nc.gpsimd.dma_start(
                        ft[:],
                        transT_d[b, h * 128:(h + 1) * 128, i0:i0 + ib, :],
                    )
                    tt = stream_pool.tile([128, ib, N], BF16)
                    nc.vector.tensor_copy(tt[:], ft[:])
                    ch.append(tt)
                matvec_block(psum_b, b, ch, i0, ib)
            return psum_b

        def epilogue(b, psum_b):
            # P0[j,i] = exp(rel[j,i])  (rel bounded ~|6|, no max-shift needed)
            P0 = epi_pool.tile([N, N], F32)
            nc.scalar.activation(P0[:], psum_b[:], ActF.Exp, scale=1.0)
            # P = P0 * mask; S[j] = sum_i P[j,i]  (one DVE op)
            P = epi_pool.tile([N, N], F32)
            S = epi_pool.tile([N, 1], F32)
            nc.vector.scalar_tensor_tensor(
                out=P[:], in0=P0[:], scalar=1.0,
                in1=pB[0:N, B_MASK + b * N:B_MASK + (b + 1) * N],
                op0=Alu.mult, op1=Alu.mult, accum_out=S[:],
            )
            Sinv = epi_pool.tile([N, 1], F32)
            nc.vector.reciprocal(Sinv[:], S[:])
            w_sb = epi_pool.tile([N, 1], F32)
            nc.vector.tensor_mul(w_sb[:], pB[0:N, B_Z + b:B_Z + b + 1], Sinv[:])
            # out[i] = sum_j P[j,i] * w[j]
            o_ps = psum_sm.tile([N, 1], F32)
            nc.tensor.matmul(o_ps[:], P[:], w_sb[:], start=True, stop=True)
            nc.scalar.copy(outT_sb[:, b:b + 1], o_ps[:])

        for b in range(BPC):
            psum_b = sample(b)
            epilogue(b, psum_b)

        nc.sync.dma_start(outT_d[:], outT_sb[:])

    nc.compile()
    return nc


_nc_cache = None


def _get_nc():
    global _nc_cache
    if _nc_cache is None:
        _nc_cache = _build()
    return _nc_cache


def _make_in_maps(q, trans_mat, r_mask, z_logits, W_r, b_r, W_q, b_q, W_out, b_out):
    bf16 = ml_dtypes.bfloat16
    in_maps = []
    transT = np.ascontiguousarray(trans_mat.transpose(0, 3, 1, 2))
    Wqpk = W_q.reshape(HK, 128, MID).transpose(1, 0, 2).reshape(128, HK * MID)
    Wrpk = W_r.T.reshape(MK, 128, E).transpose(1, 0, 2).reshape(128, MK * E)
    bw = np.concatenate([b_q.reshape(MK, 128).T, W_out.reshape(MK, 128).T], axis=1)
    for c in range(NCORES):
        b0 = c * BPC
        qpk = (q[b0:b0 + BPC].T.reshape(HK, 128, BPC)
               .transpose(1, 0, 2).reshape(128, HK * BPC))
        packA = np.concatenate([Wqpk, Wrpk, qpk], axis=1).astype(bf16)
        packB = np.zeros((128, B_W), dtype=np.float32)
        packB[:, B_BW:B_BW + 2 * MK] = bw
        packB[0:N, B_Z:B_Z + BPC] = z_logits[b0:b0 + BPC].T
        packB[0:N, B_MASK:] = (
            r_mask[b0:b0 + BPC].transpose(2, 0, 1).reshape(N, BPC * N)
            .astype(np.float32))
        in_maps.append({
            "transT": transT[b0:b0 + BPC],
            "packA": np.ascontiguousarray(packA),
            "packB": packB,
        })
    return in_maps


def _run(inputs, trace=False, **kwargs):
    nc = _get_nc()
    in_maps = _make_in_maps(**inputs)
    res = run_bass_kernel_spmd(nc, in_maps, list(range(NCORES)),
                               trace=trace, **kwargs)
    out = np.empty((B, N), dtype=np.float32)
    for c in range(NCORES):
        out[c * BPC:(c + 1) * BPC, :] = np.asarray(res.results[c]["outT"]).T
    return out, res


def kernel(**inputs):
    out, _ = _run(inputs)
    return out


# revision 28
# speedup vs baseline: 1.0205x; 1.0205x over previous
"""Trainium2 Bass kernel for nn_Net_76270029242478 (gnn_message_passing).

Math (B=32, N=100, E=256, H=1024, MID=256):
  t        = einsum('bije,em->bijm', trans_mat, W_r) + b_r
  qp       = q @ W_q + b_q
  relation = einsum('bijm,m->bij', t * qp[:,None,None,:], W_out[:,0]) + b_out
  relation = where(r_mask==0, -inf, relation); softmax over i (axis=1)
  out      = einsum('bij,bj->bi', softmax, z_logits)

Algebraic fold (exact):
  relation[b,i,j] = trans_mat[b,i,j,:] . u[b,:] + c[b]
    u[b,e] = sum_m W_r[e,m] * (qp[b,m]+b_q[m]) * W_out[m,0]
  c[b] is constant over (i,j) so it cancels in the softmax over i.

Device strategy (v5):
  - Host pre-transposes trans_mat to [b, e, i, j]: the device streams it with
    e on partitions as fully contiguous 4-20 KB descriptor runs across all
    128 partitions -> all 16 SDMA engines at the ~358 GB/s HBM roofline.
  - Stream DMAs cast f32 -> bf16 in flight (SWDGE/gpsimd) so PE weight loads
    run at 1 cycle/column (fp32 would be 4).
  - ALL small tensors (weights, q, biases, z, f32-converted mask) are packed
    on the host into TWO [128, X] blocks loaded with two DMAs with fat
    descriptors: they land in a couple of us even while the stream hogs the
    SDMA engines, and no device-side cast/unpack op exists to tangle the
    engine programs.
  - The first i-rows of sample 0 are loaded as f32 on the HWDGE rings (ready
    ~2 us before the SWDGE path) and cast on the (otherwise idle) DVE,
    hiding the SWDGE warmup.
  - rel is computed on the TensorEngine as a batched mat-vec over e:
    psum[j, i] += T[e, i, j]^T u[b, e], two 128-e halves per column.
  - Softmax lands in [j_part, i_free] layout: exp (ACT), mask-mult +
    denominator (one DVE op with accum), final aggregation is one matmul.
  - The last sample's chunks taper so the post-stream tail is short.

Sharding: data-parallel over batch, 4 samples per core x 8 cores.
"""

import ml_dtypes
import numpy as np

import concourse.bass as bass
import concourse.tile as tile
from concourse import bacc, mybir
from concourse.bass_utils import run_bass_kernel_spmd

F32 = mybir.dt.float32
BF16 = mybir.dt.bfloat16
Alu = mybir.AluOpType
ActF = mybir.ActivationFunctionType

B, N, E, H, MID = 32, 100, 256, 1024, 256
NCORES = 8
BPC = B // NCORES       # samples per core = 4
EH = E // 128           # 2 e-halves (contraction chunks)
HK = H // 128           # 8 contraction chunks for q @ W_q
MK = MID // 128         # 2 contraction chunks
WARM = 10               # i-rows of sample 0 loaded via HWDGE f32 warm-start
# i-row chunk schedule per sample (SWDGE bf16 cast stream)
CHUNKS = {
    0: [(WARM, 20), (30, 20), (50, 25), (75, 25)],  # 0:10 = the warm pair
    1: [(0, 25), (25, 25), (50, 25), (75, 25)],
    2: [(0, 25), (25, 25), (50, 25), (75, 25)],
    # taper the last sample so the post-stream cast+matvec drain is short
    3: [(0, 25), (25, 25), (50, 25), (75, 15), (90, 10)],
}
# packA (bf16) column offsets
A_WQ, A_WR, A_Q = 0, HK * MID, HK * MID + MK * E
A_W = A_Q + HK * BPC                     # 2592
# packB (f32) column offsets
B_BW, B_Z, B_MASK = 0, 2 * MK, 2 * MK + BPC
B_W = B_MASK + BPC * N                   # 408


def _build():
    nc = bacc.Bacc("TRN2", target_bir_lowering=False, debug=False,
                   num_devices=NCORES)

    # trans pre-transposed on host to [b, e, i, j] (e on partitions)
    transT_d = nc.declare_dram_parameter("transT", [BPC, E, N, N], F32,
                                         isOutput=False)
    packA_d = nc.declare_dram_parameter("packA", [128, A_W], BF16, isOutput=False)
    packB_d = nc.declare_dram_parameter("packB", [128, B_W], F32, isOutput=False)
    outT_d = nc.declare_dram_parameter("outT", [N, BPC], F32, isOutput=True)

    with tile.TileContext(nc) as tc, \
         tc.tile_pool(name="const", bufs=1) as const_pool, \
         tc.tile_pool(name="stream", bufs=8) as stream_pool, \
         tc.tile_pool(name="swf32", bufs=6) as swf32_pool, \
         tc.tile_pool(name="warm", bufs=2) as warm_pool, \
         tc.tile_pool(name="epi", bufs=6) as epi_pool, \
         tc.tile_pool(name="psum_rel", bufs=2, space="PSUM") as psum_rel, \
         tc.tile_pool(name="psum_sm", bufs=2, space="PSUM") as psum_sm:

        # ---------- consts first (fat descriptors, land in ~2-3 us) --------
        pA = const_pool.tile([128, A_W], BF16)
        nc.sync.dma_start(pA[:], packA_d[:])
        pB = const_pool.tile([128, B_W], F32)
        nc.scalar.dma_start(pB[:], packB_d[:])

        # ---------- warm-start: rows 0:WARM of sample 0 as f32 on HWDGE ----
        warm_f32 = []
        for h in range(EH):
            wt = warm_pool.tile([128, WARM, N], F32)
            eng = nc.sync if h == 0 else nc.scalar
            eng.dma_start(wt[:], transT_d[0, h * 128:(h + 1) * 128, 0:WARM, :])
            warm_f32.append(wt)
        warm_bf = []
        for h in range(EH):
            wb = stream_pool.tile([128, WARM, N], BF16)
            nc.vector.tensor_copy(wb[:], warm_f32[h][:])
            warm_bf.append(wb)

        # ---------- prologue: u[b,e] with e on partitions, bf16 ----------
        # qpT[m,b] = sum_h W_q[h,m] * q[b,h]
        vT_sb = const_pool.tile([128, MK, BPC], BF16)
        for mk in range(MK):
            qpT_ps = psum_sm.tile([128, BPC], F32)
            for hk in range(HK):
                nc.tensor.matmul(
                    qpT_ps[:],
                    pA[:, A_WQ + hk * MID + mk * 128:A_WQ + hk * MID + (mk + 1) * 128],
                    pA[:, A_Q + hk * BPC:A_Q + (hk + 1) * BPC],
                    start=(hk == 0), stop=(hk == HK - 1),
                )
            # vT[m,b] = (qpT[m,b] + b_q[m]) * W_out[m]
            nc.vector.tensor_scalar(
                out=vT_sb[:, mk, :], in0=qpT_ps[:],
                scalar1=pB[:, B_BW + mk:B_BW + mk + 1],
                scalar2=pB[:, B_BW + MK + mk:B_BW + MK + mk + 1],
                op0=Alu.add, op1=Alu.mult,
            )

        # uT[e', h, b] = sum_m W_r[128h+e', m] * vT[m, b]  (partition = e')
        uTb_sb = const_pool.tile([128, EH, BPC], BF16)
        for h in range(EH):
            uT_ps = psum_sm.tile([128, BPC], F32)
            for mk in range(MK):
                nc.tensor.matmul(
                    uT_ps[:],
                    pA[:, A_WR + mk * E + h * 128:A_WR + mk * E + (h + 1) * 128],
                    vT_sb[:, mk, :],
                    start=(mk == 0), stop=(mk == MK - 1),
                )
            nc.vector.tensor_copy(uTb_sb[:, h, :], uT_ps[:])

        outT_sb = const_pool.tile([N, BPC], F32)

        # ---------- main stream + batched mat-vec on the PE ----------
        def matvec_block(psum_b, b, ch_pair, i0, ib):
            for il in range(ib):
                i = i0 + il
                nc.tensor.matmul(psum_b[:, i:i + 1], ch_pair[0][:, il, :],
                                 uTb_sb[:, 0, b:b + 1], start=True, stop=False)
                nc.tensor.matmul(psum_b[:, i:i + 1], ch_pair[1][:, il, :],
                                 uTb_sb[:, 1, b:b + 1], start=False, stop=True)

        def sample(b):
            psum_b = psum_rel.tile([N, N], F32)
            if b == 0:
                matvec_block(psum_b, 0, warm_bf, 0, WARM)
            for (i0, ib) in CHUNKS[b]:
                ch = []
                for h in range(EH):
                    # SWDGE f32 load (no in-DMA cast), bf16 cast on the DVE
                    ft = swf32_pool.tile([128, ib, N], F32)
                    nc.gpsimd.dma_start(
                        ft[:],
                        transT_d[b, h * 128:(h + 1) * 128, i0:i0 + ib, :],
                    )
                    tt = stream_pool.tile([128, ib, N], BF16)
                    nc.vector.tensor_copy(tt[:], ft[:])
                    ch.append(tt)
                matvec_block(psum_b, b, ch, i0, ib)
            return psum_b

        def epilogue(b, psum_b):
            # P0[j,i] = exp(rel[j,i])  (rel bounded ~|6|, no max-shift needed)
            P0 = epi_pool.tile([N, N], F32)
            nc.scalar.activation(P0[:], psum_b[:], ActF.Exp, scale=1.0)
            # P = P0 * mask; S[j] = sum_i P[j,i]  (one DVE op)
            P = epi_pool.tile([N, N], F32)
            S = epi_pool.tile([N, 1], F32)
            nc.vector.scalar_tensor_tensor(
                out=P[:], in0=P0[:], scalar=1.0,
                in1=pB[0:N, B_MASK + b * N:B_MASK + (b + 1) * N],
                op0=Alu.mult, op1=Alu.mult, accum_out=S[:],
            )
            Sinv = epi_pool.tile([N, 1], F32)
            nc.vector.reciprocal(Sinv[:], S[:])
            w_sb = epi_pool.tile([N, 1], F32)
            nc.vector.tensor_mul(w_sb[:], pB[0:N, B_Z + b:B_Z + b + 1], Sinv[:])
            # out[i] = sum_j P[j,i] * w[j]
            o_ps = psum_sm.tile([N, 1], F32)
            nc.tensor.matmul(o_ps[:], P[:], w_sb[:], start=True, stop=True)
            nc.scalar.copy(outT_sb[:, b:b + 1], o_ps[:])

        for b in range(BPC):
            psum_b = sample(b)
            epilogue(b, psum_b)

        nc.sync.dma_start(outT_d[:], outT_sb[:])

    nc.compile()
    return nc


_nc_cache = None


def _get_nc():
    global _nc_cache
    if _nc_cache is None:
        _nc_cache = _build()
    return _nc_cache


def _make_in_maps(q, trans_mat, r_mask, z_logits, W_r, b_r, W_q, b_q, W_out, b_out):
    bf16 = ml_dtypes.bfloat16
    in_maps = []
    transT = np.ascontiguousarray(trans_mat.transpose(0, 3, 1, 2))
    Wqpk = W_q.reshape(HK, 128, MID).transpose(1, 0, 2).reshape(128, HK * MID)
    Wrpk = W_r.T.reshape(MK, 128, E).transpose(1, 0, 2).reshape(128, MK * E)
    bw = np.concatenate([b_q.reshape(MK, 128).T, W_out.reshape(MK, 128).T], axis=1)
    for c in range(NCORES):
        b0 = c * BPC
        qpk = (q[b0:b0 + BPC].T.reshape(HK, 128, BPC)
               .transpose(1, 0, 2).reshape(128, HK * BPC))
        packA = np.concatenate([Wqpk, Wrpk, qpk], axis=1).astype(bf16)
        packB = np.zeros((128, B_W), dtype=np.float32)
        packB[:, B_BW:B_BW + 2 * MK] = bw
        packB[0:N, B_Z:B_Z + BPC] = z_logits[b0:b0 + BPC].T
        packB[0:N, B_MASK:] = (
            r_mask[b0:b0 + BPC].transpose(2, 0, 1).reshape(N, BPC * N)
            .astype(np.float32))
        in_maps.append({
            "transT": transT[b0:b0 + BPC],
            "packA": np.ascontiguousarray(packA),
            "packB": packB,
        })
    return in_maps


def _run(inputs, trace=False, **kwargs):
    nc = _get_nc()
    in_maps = _make_in_maps(**inputs)
    res = run_bass_kernel_spmd(nc, in_maps, list(range(NCORES)),
                               trace=trace, **kwargs)
    out = np.empty((B, N), dtype=np.float32)
    for c in range(NCORES):
        out[c * BPC:(c + 1) * BPC, :] = np.asarray(res.results[c]["outT"]).T
    return out, res


def kernel(**inputs):
    out, _ = _run(inputs)
    return out


# revision 29
# speedup vs baseline: 1.0664x; 1.0450x over previous
"""Trainium2 Bass kernel for nn_Net_76270029242478 (gnn_message_passing).

Math (B=32, N=100, E=256, H=1024, MID=256):
  t        = einsum('bije,em->bijm', trans_mat, W_r) + b_r
  qp       = q @ W_q + b_q
  relation = einsum('bijm,m->bij', t * qp[:,None,None,:], W_out[:,0]) + b_out
  relation = where(r_mask==0, -inf, relation); softmax over i (axis=1)
  out      = einsum('bij,bj->bi', softmax, z_logits)

Algebraic fold (exact):
  relation[b,i,j] = trans_mat[b,i,j,:] . u[b,:] + c[b]
    u[b,e] = sum_m W_r[e,m] * (qp[b,m]+b_q[m]) * W_out[m,0]
  c[b] is constant over (i,j) so it cancels in the softmax over i.

Device strategy (v5):
  - Host pre-transposes trans_mat to [b, e, i, j]: the device streams it with
    e on partitions as fully contiguous 4-20 KB descriptor runs across all
    128 partitions -> all 16 SDMA engines at the ~358 GB/s HBM roofline.
  - Stream DMAs cast f32 -> bf16 in flight (SWDGE/gpsimd) so PE weight loads
    run at 1 cycle/column (fp32 would be 4).
  - ALL small tensors (weights, q, biases, z, f32-converted mask) are packed
    on the host into TWO [128, X] blocks loaded with two DMAs with fat
    descriptors: they land in a couple of us even while the stream hogs the
    SDMA engines, and no device-side cast/unpack op exists to tangle the
    engine programs.
  - The first i-rows of sample 0 are loaded as f32 on the HWDGE rings (ready
    ~2 us before the SWDGE path) and cast on the (otherwise idle) DVE,
    hiding the SWDGE warmup.
  - rel is computed on the TensorEngine as a batched mat-vec over e:
    psum[j, i] += T[e, i, j]^T u[b, e], two 128-e halves per column.
  - Softmax lands in [j_part, i_free] layout: exp (ACT), mask-mult +
    denominator (one DVE op with accum), final aggregation is one matmul.
  - The last sample's chunks taper so the post-stream tail is short.

Sharding: data-parallel over batch, 4 samples per core x 8 cores.
"""

import ml_dtypes
import numpy as np

import concourse.bass as bass
import concourse.tile as tile
from concourse import bacc, mybir
from concourse.bass_utils import run_bass_kernel_spmd

F32 = mybir.dt.float32
BF16 = mybir.dt.bfloat16
Alu = mybir.AluOpType
ActF = mybir.ActivationFunctionType

B, N, E, H, MID = 32, 100, 256, 1024, 256
NCORES = 8
BPC = B // NCORES       # samples per core = 4
EH = E // 128           # 2 e-halves (contraction chunks)
HK = H // 128           # 8 contraction chunks for q @ W_q
MK = MID // 128         # 2 contraction chunks
WARM = 10               # i-rows of sample 0 loaded via HWDGE f32 warm-start
# i-row chunk schedule per sample (SWDGE bf16 cast stream)
CHUNKS = {
    0: [(WARM, 20), (30, 20), (50, 25), (75, 25)],  # 0:10 = the warm pair
    1: [(0, 25), (25, 25), (50, 25), (75, 25)],
    2: [(0, 25), (25, 25), (50, 25), (75, 25)],
    # taper the last sample so the post-stream cast+matvec drain is short
    3: [(0, 25), (25, 25), (50, 25), (75, 15), (90, 10)],
}
# packA (bf16) column offsets
A_WQ, A_WR, A_Q = 0, HK * MID, HK * MID + MK * E
A_W = A_Q + HK * BPC                     # 2592
# packB (f32) column offsets
B_BW, B_Z, B_MASK = 0, 2 * MK, 2 * MK + BPC
B_W = B_MASK + BPC * N                   # 408


def _build():
    nc = bacc.Bacc("TRN2", target_bir_lowering=False, debug=False,
                   num_devices=NCORES)

    # trans pre-transposed on host to [b, e, i, j] (e on partitions)
    transT_d = nc.declare_dram_parameter("transT", [BPC, E, N, N], F32,
                                         isOutput=False)
    packA_d = nc.declare_dram_parameter("packA", [128, A_W], BF16, isOutput=False)
    packB_d = nc.declare_dram_parameter("packB", [128, B_W], F32, isOutput=False)
    outT_d = nc.declare_dram_parameter("outT", [N, BPC], F32, isOutput=True)

    with tile.TileContext(nc) as tc, \
         tc.tile_pool(name="const", bufs=1) as const_pool, \
         tc.tile_pool(name="stream", bufs=8) as stream_pool, \
         tc.tile_pool(name="swf32", bufs=6) as swf32_pool, \
         tc.tile_pool(name="warm", bufs=2) as warm_pool, \
         tc.tile_pool(name="epi", bufs=6) as epi_pool, \
         tc.tile_pool(name="psum_rel", bufs=2, space="PSUM") as psum_rel, \
         tc.tile_pool(name="psum_sm", bufs=2, space="PSUM") as psum_sm:

        # ---------- consts first (fat descriptors, land in ~2-3 us) --------
        pA = const_pool.tile([128, A_W], BF16)
        nc.sync.dma_start(pA[:], packA_d[:])
        pB = const_pool.tile([128, B_W], F32)
        nc.scalar.dma_start(pB[:], packB_d[:])

        # ---------- warm-start: rows 0:WARM of sample 0 as f32 on HWDGE ----
        warm_f32 = []
        for h in range(EH):
            wt = warm_pool.tile([128, WARM, N], F32)
            eng = nc.sync if h == 0 else nc.scalar
            eng.dma_start(wt[:], transT_d[0, h * 128:(h + 1) * 128, 0:WARM, :])
            warm_f32.append(wt)
        warm_bf = []
        for h in range(EH):
            wb = stream_pool.tile([128, WARM, N], BF16)
            nc.vector.tensor_copy(wb[:], warm_f32[h][:])
            warm_bf.append(wb)

        # ---------- prologue: u[b,e] with e on partitions, bf16 ----------
        # qpT[m,b] = sum_h W_q[h,m] * q[b,h]
        vT_sb = const_pool.tile([128, MK, BPC], BF16)
        for mk in range(MK):
            qpT_ps = psum_sm.tile([128, BPC], F32)
            for hk in range(HK):
                nc.tensor.matmul(
                    qpT_ps[:],
                    pA[:, A_WQ + hk * MID + mk * 128:A_WQ + hk * MID + (mk + 1) * 128],
                    pA[:, A_Q + hk * BPC:A_Q + (hk + 1) * BPC],
                    start=(hk == 0), stop=(hk == HK - 1),
                )
            # vT[m,b] = (qpT[m,b] + b_q[m]) * W_out[m]
            nc.vector.tensor_scalar(
                out=vT_sb[:, mk, :], in0=qpT_ps[:],
                scalar1=pB[:, B_BW + mk:B_BW + mk + 1],
                scalar2=pB[:, B_BW + MK + mk:B_BW + MK + mk + 1],
                op0=Alu.add, op1=Alu.mult,
            )

        # uT[e', h, b] = sum_m W_r[128h+e', m] * vT[m, b]  (partition = e')
        uTb_sb = const_pool.tile([128, EH, BPC], BF16)
        for h in range(EH):
            uT_ps = psum_sm.tile([128, BPC], F32)
            for mk in range(MK):
                nc.tensor.matmul(
                    uT_ps[:],
                    pA[:, A_WR + mk * E + h * 128:A_WR + mk * E + (h + 1) * 128],
                    vT_sb[:, mk, :],
                    start=(mk == 0), stop=(mk == MK - 1),
                )
            nc.vector.tensor_copy(uTb_sb[:, h, :], uT_ps[:])

        outT_sb = const_pool.tile([N, BPC], F32)

        # ---------- main stream + batched mat-vec on the PE ----------
        def matvec_block(psum_b, b, ch_pair, i0, ib):
            for il in range(ib):
                i = i0 + il
                nc.tensor.matmul(psum_b[:, i:i + 1], ch_pair[0][:, il, :],
                                 uTb_sb[:, 0, b:b + 1], start=True, stop=False)
                nc.tensor.matmul(psum_b[:, i:i + 1], ch_pair[1][:, il, :],
                                 uTb_sb[:, 1, b:b + 1], start=False, stop=True)

        def sample(b):
            psum_b = psum_rel.tile([N, N], F32)
            if b == 0:
                matvec_block(psum_b, 0, warm_bf, 0, WARM)
            for (i0, ib) in CHUNKS[b]:
                ch = []
                for h in range(EH):
                    # SWDGE f32 load (no in-DMA cast), bf16 cast on the DVE
                    ft = swf32_pool.tile([128, ib, N], F32)
                    nc.gpsimd.dma_start(
                        ft[:],
                        transT_d[b, h * 128:(h + 1) * 128, i0:i0 + ib, :],
                    )
                    tt = stream_pool.tile([128, ib, N], BF16)
                    nc.vector.tensor_copy(tt[:], ft[:])
                    ch.append(tt)
                matvec_block(psum_b, b, ch, i0, ib)
            return psum_b

        def epilogue(b, psum_b):
            # P0[j,i] = exp(rel[j,i])  (rel bounded ~|6|, no max-shift needed)
            P0 = epi_pool.tile([N, N], F32)
            nc.scalar.activation(P0[:], psum_b[:], ActF.Exp, scale=1.0)
            # P = P0 * mask; S[j] = sum_i P[j,i]  (one DVE op)
            P = epi_pool.tile([N, N], F32)
            S = epi_pool.tile([N, 1], F32)
            nc.vector.scalar_tensor_tensor(
                out=P[:], in0=P0[:], scalar=1.0,
                in1=pB[0:N, B_MASK + b * N:B_MASK + (b + 1) * N],
                op0=Alu.mult, op1=Alu.mult, accum_out=S[:],
            )
            Sinv = epi_pool.tile([N, 1], F32)
            nc.vector.reciprocal(Sinv[:], S[:])
            w_sb = epi_pool.tile([N, 1], F32)
            nc.vector.tensor_mul(w_sb[:], pB[0:N, B_Z + b:B_Z + b + 1], Sinv[:])
            # out[i] = sum_j P[j,i] * w[j]
            o_ps = psum_sm.tile([N, 1], F32)
            nc.tensor.matmul(o_ps[:], P[:], w_sb[:], start=True, stop=True)
            nc.scalar.copy(outT_sb[:, b:b + 1], o_ps[:])

        # delay each epilogue by one sample: the next sample's DVE casts must
        # not queue behind the epilogue's DVE ops, or the stream stalls at
        # every sample boundary
        ps_prev = None
        for b in range(BPC):
            psum_b = sample(b)
            if ps_prev is not None:
                epilogue(b - 1, ps_prev)
            ps_prev = psum_b
        epilogue(BPC - 1, ps_prev)

        nc.sync.dma_start(outT_d[:], outT_sb[:])

    nc.compile()
    return nc


_nc_cache = None


def _get_nc():
    global _nc_cache
    if _nc_cache is None:
        _nc_cache = _build()
    return _nc_cache


def _make_in_maps(q, trans_mat, r_mask, z_logits, W_r, b_r, W_q, b_q, W_out, b_out):
    bf16 = ml_dtypes.bfloat16
    in_maps = []
    transT = np.ascontiguousarray(trans_mat.transpose(0, 3, 1, 2))
    Wqpk = W_q.reshape(HK, 128, MID).transpose(1, 0, 2).reshape(128, HK * MID)
    Wrpk = W_r.T.reshape(MK, 128, E).transpose(1, 0, 2).reshape(128, MK * E)
    bw = np.concatenate([b_q.reshape(MK, 128).T, W_out.reshape(MK, 128).T], axis=1)
    for c in range(NCORES):
        b0 = c * BPC
        qpk = (q[b0:b0 + BPC].T.reshape(HK, 128, BPC)
               .transpose(1, 0, 2).reshape(128, HK * BPC))
        packA = np.concatenate([Wqpk, Wrpk, qpk], axis=1).astype(bf16)
        packB = np.zeros((128, B_W), dtype=np.float32)
        packB[:, B_BW:B_BW + 2 * MK] = bw
        packB[0:N, B_Z:B_Z + BPC] = z_logits[b0:b0 + BPC].T
        packB[0:N, B_MASK:] = (
            r_mask[b0:b0 + BPC].transpose(2, 0, 1).reshape(N, BPC * N)
            .astype(np.float32))
        in_maps.append({
            "transT": transT[b0:b0 + BPC],
            "packA": np.ascontiguousarray(packA),
            "packB": packB,
        })
    return in_maps


def _run(inputs, trace=False, **kwargs):
    nc = _get_nc()
    in_maps = _make_in_maps(**inputs)
    res = run_bass_kernel_spmd(nc, in_maps, list(range(NCORES)),
                               trace=trace, **kwargs)
    out = np.empty((B, N), dtype=np.float32)
    for c in range(NCORES):
        out[c * BPC:(c + 1) * BPC, :] = np.asarray(res.results[c]["outT"]).T
    return out, res


def kernel(**inputs):
    out, _ = _run(inputs)
    return out


# revision 30
# speedup vs baseline: 1.1229x; 1.0530x over previous
"""Trainium2 Bass kernel for nn_Net_76270029242478 (gnn_message_passing).

Math (B=32, N=100, E=256, H=1024, MID=256):
  t        = einsum('bije,em->bijm', trans_mat, W_r) + b_r
  qp       = q @ W_q + b_q
  relation = einsum('bijm,m->bij', t * qp[:,None,None,:], W_out[:,0]) + b_out
  relation = where(r_mask==0, -inf, relation); softmax over i (axis=1)
  out      = einsum('bij,bj->bi', softmax, z_logits)

Algebraic fold (exact):
  relation[b,i,j] = trans_mat[b,i,j,:] . u[b,:] + c[b]
    u[b,e] = sum_m W_r[e,m] * (qp[b,m]+b_q[m]) * W_out[m,0]
  c[b] is constant over (i,j) so it cancels in the softmax over i.

Device strategy:
  - Host pre-transposes trans_mat to [b, e, i, j]: the device streams it with
    e on partitions as fully contiguous ~10 KB descriptor runs across all
    128 partitions -> all 16 SDMA engines at the ~358 GB/s HBM roofline.
    The stream runs on the SWDGE (gpsimd) queue as plain f32 (an in-DMA
    bf16 cast throttles the SDMA datapath ~10%); the otherwise-idle DVE
    downcasts each chunk to bf16 so PE weight loads run at 1 cycle/column
    (fp32 weights would load at 4 cycles/column).
  - ALL small tensors (weights, q, biases, z, f32-converted mask) are packed
    on the host into TWO [128, X] blocks loaded with two DMAs with fat
    descriptors: they land in a couple of us even while the stream hogs the
    SDMA engines, and no device-side cast/unpack op exists to tangle the
    engine programs.
  - The first i-rows of sample 0 are loaded as f32 on the HWDGE rings (ready
    ~2 us before the SWDGE path), hiding the SWDGE warmup.
  - rel is computed on the TensorEngine as a batched mat-vec over e:
    psum[j, i] += T[e, i, j]^T u[b, e], two 128-e halves per column.
  - Softmax lands in [j_part, i_free] layout: exp (ACT), mask-mult +
    denominator (one DVE op with accum), final aggregation is one matmul.
  - The last sample's chunks taper so the post-stream tail is short.

Sharding: data-parallel over batch, 4 samples per core x 8 cores.
"""

import ml_dtypes
import numpy as np

import concourse.bass as bass
import concourse.tile as tile
from concourse import bacc, mybir
from concourse.bass_utils import run_bass_kernel_spmd

F32 = mybir.dt.float32
BF16 = mybir.dt.bfloat16
Alu = mybir.AluOpType
ActF = mybir.ActivationFunctionType

B, N, E, H, MID = 32, 100, 256, 1024, 256
NCORES = 8
BPC = B // NCORES       # samples per core = 4
EH = E // 128           # 2 e-halves (contraction chunks)
HK = H // 128           # 8 contraction chunks for q @ W_q
MK = MID // 128         # 2 contraction chunks
WARM = 10               # i-rows of sample 0 loaded via HWDGE f32 warm-start
# i-row chunk schedule per sample (SWDGE bf16 cast stream)
CHUNKS = {
    0: [(WARM, 20), (30, 20), (50, 25), (75, 25)],  # 0:10 = the warm pair
    1: [(0, 25), (25, 25), (50, 25), (75, 25)],
    2: [(0, 25), (25, 25), (50, 25), (75, 25)],
    # taper the last sample so the post-stream cast+matvec drain is short
    3: [(0, 25), (25, 25), (50, 25), (75, 15), (90, 10)],
}
# packA (bf16) column offsets
A_WQ, A_WR, A_Q = 0, HK * MID, HK * MID + MK * E
A_W = A_Q + HK * BPC                     # 2592
# packB (f32) column offsets
B_BW, B_Z, B_MASK = 0, 2 * MK, 2 * MK + BPC
B_W = B_MASK + BPC * N                   # 408


def _build():
    nc = bacc.Bacc("TRN2", target_bir_lowering=False, debug=False,
                   num_devices=NCORES)

    # trans pre-transposed on host to [b, e, i, j] (e on partitions)
    transT_d = nc.declare_dram_parameter("transT", [BPC, E, N, N], F32,
                                         isOutput=False)
    packA_d = nc.declare_dram_parameter("packA", [128, A_W], BF16, isOutput=False)
    packB_d = nc.declare_dram_parameter("packB", [128, B_W], F32, isOutput=False)
    outT_d = nc.declare_dram_parameter("outT", [N, BPC], F32, isOutput=True)

    with tile.TileContext(nc) as tc, \
         tc.tile_pool(name="const", bufs=1) as const_pool, \
         tc.tile_pool(name="stream", bufs=8) as stream_pool, \
         tc.tile_pool(name="swf32", bufs=6) as swf32_pool, \
         tc.tile_pool(name="warm", bufs=2) as warm_pool, \
         tc.tile_pool(name="epi", bufs=6) as epi_pool, \
         tc.tile_pool(name="psum_rel", bufs=2, space="PSUM") as psum_rel, \
         tc.tile_pool(name="psum_sm", bufs=2, space="PSUM") as psum_sm:

        # ---------- consts first (fat descriptors, land in ~2-3 us) --------
        pA = const_pool.tile([128, A_W], BF16)
        nc.sync.dma_start(pA[:], packA_d[:])
        pB = const_pool.tile([128, B_W], F32)
        nc.scalar.dma_start(pB[:], packB_d[:])

        # ---------- warm-start: rows 0:WARM of sample 0 as f32 on HWDGE ----
        warm_f32 = []
        for h in range(EH):
            wt = warm_pool.tile([128, WARM, N], F32)
            eng = nc.sync if h == 0 else nc.scalar
            eng.dma_start(wt[:], transT_d[0, h * 128:(h + 1) * 128, 0:WARM, :])
            warm_f32.append(wt)
        warm_bf = []
        for h in range(EH):
            wb = stream_pool.tile([128, WARM, N], BF16)
            nc.vector.tensor_copy(wb[:], warm_f32[h][:])
            warm_bf.append(wb)

        # ---------- prologue: u[b,e] with e on partitions, bf16 ----------
        # qpT[m,b] = sum_h W_q[h,m] * q[b,h]
        vT_sb = const_pool.tile([128, MK, BPC], BF16)
        for mk in range(MK):
            qpT_ps = psum_sm.tile([128, BPC], F32)
            for hk in range(HK):
                nc.tensor.matmul(
                    qpT_ps[:],
                    pA[:, A_WQ + hk * MID + mk * 128:A_WQ + hk * MID + (mk + 1) * 128],
                    pA[:, A_Q + hk * BPC:A_Q + (hk + 1) * BPC],
                    start=(hk == 0), stop=(hk == HK - 1),
                )
            # vT[m,b] = (qpT[m,b] + b_q[m]) * W_out[m]
            nc.vector.tensor_scalar(
                out=vT_sb[:, mk, :], in0=qpT_ps[:],
                scalar1=pB[:, B_BW + mk:B_BW + mk + 1],
                scalar2=pB[:, B_BW + MK + mk:B_BW + MK + mk + 1],
                op0=Alu.add, op1=Alu.mult,
            )

        # uT[e', h, b] = sum_m W_r[128h+e', m] * vT[m, b]  (partition = e')
        uTb_sb = const_pool.tile([128, EH, BPC], BF16)
        for h in range(EH):
            uT_ps = psum_sm.tile([128, BPC], F32)
            for mk in range(MK):
                nc.tensor.matmul(
                    uT_ps[:],
                    pA[:, A_WR + mk * E + h * 128:A_WR + mk * E + (h + 1) * 128],
                    vT_sb[:, mk, :],
                    start=(mk == 0), stop=(mk == MK - 1),
                )
            nc.vector.tensor_copy(uTb_sb[:, h, :], uT_ps[:])

        outT_sb = const_pool.tile([N, BPC], F32)

        # ---------- main stream + batched mat-vec on the PE ----------
        def matvec_block(psum_b, b, ch_pair, i0, ib):
            for il in range(ib):
                i = i0 + il
                nc.tensor.matmul(psum_b[:, i:i + 1], ch_pair[0][:, il, :],
                                 uTb_sb[:, 0, b:b + 1], start=True, stop=False)
                nc.tensor.matmul(psum_b[:, i:i + 1], ch_pair[1][:, il, :],
                                 uTb_sb[:, 1, b:b + 1], start=False, stop=True)

        def sample(b):
            psum_b = psum_rel.tile([N, N], F32)
            if b == 0:
                matvec_block(psum_b, 0, warm_bf, 0, WARM)
            for (i0, ib) in CHUNKS[b]:
                ch = []
                for h in range(EH):
                    # SWDGE f32 load (no in-DMA cast), bf16 cast on the DVE
                    ft = swf32_pool.tile([128, ib, N], F32)
                    nc.gpsimd.dma_start(
                        ft[:],
                        transT_d[b, h * 128:(h + 1) * 128, i0:i0 + ib, :],
                    )
                    tt = stream_pool.tile([128, ib, N], BF16)
                    nc.vector.tensor_copy(tt[:], ft[:])
                    ch.append(tt)
                matvec_block(psum_b, b, ch, i0, ib)
            return psum_b

        def epilogue(b, psum_b):
            # P0[j,i] = exp(rel[j,i])  (rel bounded ~|6|, no max-shift needed)
            P0 = epi_pool.tile([N, N], F32)
            nc.scalar.activation(P0[:], psum_b[:], ActF.Exp, scale=1.0)
            # P = P0 * mask; S[j] = sum_i P[j,i]  (one DVE op)
            P = epi_pool.tile([N, N], F32)
            S = epi_pool.tile([N, 1], F32)
            nc.vector.scalar_tensor_tensor(
                out=P[:], in0=P0[:], scalar=1.0,
                in1=pB[0:N, B_MASK + b * N:B_MASK + (b + 1) * N],
                op0=Alu.mult, op1=Alu.mult, accum_out=S[:],
            )
            Sinv = epi_pool.tile([N, 1], F32)
            nc.vector.reciprocal(Sinv[:], S[:])
            w_sb = epi_pool.tile([N, 1], F32)
            nc.vector.tensor_mul(w_sb[:], pB[0:N, B_Z + b:B_Z + b + 1], Sinv[:])
            # out[i] = sum_j P[j,i] * w[j]
            o_ps = psum_sm.tile([N, 1], F32)
            nc.tensor.matmul(o_ps[:], P[:], w_sb[:], start=True, stop=True)
            nc.scalar.copy(outT_sb[:, b:b + 1], o_ps[:])

        # delay each epilogue by one sample: the next sample's DVE casts must
        # not queue behind the epilogue's DVE ops, or the stream stalls at
        # every sample boundary
        ps_prev = None
        for b in range(BPC):
            psum_b = sample(b)
            if ps_prev is not None:
                epilogue(b - 1, ps_prev)
            ps_prev = psum_b
        epilogue(BPC - 1, ps_prev)

        nc.sync.dma_start(outT_d[:], outT_sb[:])

    nc.compile()
    return nc


_nc_cache = None


def _get_nc():
    global _nc_cache
    if _nc_cache is None:
        _nc_cache = _build()
    return _nc_cache


def _make_in_maps(q, trans_mat, r_mask, z_logits, W_r, b_r, W_q, b_q, W_out, b_out):
    bf16 = ml_dtypes.bfloat16
    in_maps = []
    transT = np.ascontiguousarray(trans_mat.transpose(0, 3, 1, 2))
    Wqpk = W_q.reshape(HK, 128, MID).transpose(1, 0, 2).reshape(128, HK * MID)
    Wrpk = W_r.T.reshape(MK, 128, E).transpose(1, 0, 2).reshape(128, MK * E)
    bw = np.concatenate([b_q.reshape(MK, 128).T, W_out.reshape(MK, 128).T], axis=1)
    for c in range(NCORES):
        b0 = c * BPC
        qpk = (q[b0:b0 + BPC].T.reshape(HK, 128, BPC)
               .transpose(1, 0, 2).reshape(128, HK * BPC))
        packA = np.concatenate([Wqpk, Wrpk, qpk], axis=1).astype(bf16)
        packB = np.zeros((128, B_W), dtype=np.float32)
        packB[:, B_BW:B_BW + 2 * MK] = bw
        packB[0:N, B_Z:B_Z + BPC] = z_logits[b0:b0 + BPC].T
        packB[0:N, B_MASK:] = (
            r_mask[b0:b0 + BPC].transpose(2, 0, 1).reshape(N, BPC * N)
            .astype(np.float32))
        in_maps.append({
            "transT": transT[b0:b0 + BPC],
            "packA": np.ascontiguousarray(packA),
            "packB": packB,
        })
    return in_maps


def _run(inputs, trace=False, **kwargs):
    nc = _get_nc()
    in_maps = _make_in_maps(**inputs)
    res = run_bass_kernel_spmd(nc, in_maps, list(range(NCORES)),
                               trace=trace, **kwargs)
    out = np.empty((B, N), dtype=np.float32)
    for c in range(NCORES):
        out[c * BPC:(c + 1) * BPC, :] = np.asarray(res.results[c]["outT"]).T
    return out, res


def kernel(**inputs):
    out, _ = _run(inputs)
    return out
